# revision 18
# baseline (speedup 1.0000x reference)
"""Trainium2 Bass kernel for the difflogic LogicLayer problem.

Computation: y = c0 + ca*a + cb*b + cab*a*b where a = x[:, idx_a],
b = x[:, idx_b] and (c0, ca, cb, cab) = softmax(weights) @ GATE_COEFS.

Strategy (8-core SPMD, data-parallel over batch):
  - Host: compute the tiny [4096, 16] softmax -> [4096, 4] coef table,
    marshal the per-core x shard as a transposed fp16 table
    xT [4096, 2048] (shard layout choice), wrap the index lists into the
    16-partition dma_gather layout. The per-core output shard is
    likewise out-dim-major yT [4096, 2048] fp16; the host unshards by
    transposing back to batch-major f32.
  - Device, per core (single pipeline):
      For each chunk of 512 output columns:
        * dma_gather rows idx_a/idx_b from DRAM xT (4 KiB fp16 rows,
          near line rate) -> out-dim-major tiles [128, 4, 2048].
        * Per 128-col slot: fused DVE tensor_scalar u = cab*b + ca,
          ACT affine v = cb*b + c0, DVE w = u*a, ys = w + v (fp16).
        * DMA ys -> yT rows (4 KiB contiguous per partition).
  HBM traffic/core: 33.5 MiB gather read + 16.8 MiB yT write.
"""
import numpy as np

import concourse.bacc as bacc
import concourse.bass as bass
import concourse.mybir as mybir
import concourse.tile as tile
from concourse.bass_utils import run_bass_kernel_spmd

# difflogic gate coefficients: rows = gates, cols = (const, a, b, ab)
GATE_COEFS = np.array([
    [0, 0, 0, 0], [0, 0, 0, 1], [0, 1, 0, -1], [0, 1, 0, 0],
    [0, 0, 1, -1], [0, 0, 1, 0], [0, 1, 1, -2], [0, 1, 1, -1],
    [1, -1, -1, 1], [1, -1, -1, 2], [1, 0, -1, 0], [1, 0, -1, 1],
    [1, -1, 0, 0], [1, -1, 0, 1], [1, 0, 0, -1], [1, 0, 0, 0],
], dtype=np.float64)  # [16, 4]

N_CORES = 8
P = 128
BATCH = 16384
IN_DIM = 4096
OUT_DIM = 4096
B = BATCH // N_CORES          # 2048 rows per core
CHUNK = 512                   # indices per dma_gather
NCH = OUT_DIM // CHUNK        # 8 chunks
SLOTS = CHUNK // P            # 4 slots per chunk
M = OUT_DIM // P              # 32 col blocks

F32 = mybir.dt.float32
F16 = mybir.dt.float16
I16 = mybir.dt.int16

LAST_EXEC_NS = None
_NC_CACHE = {}


def _build_nc():
    nc = bacc.Bacc("TRN2", target_bir_lowering=False, debug=False,
                   num_devices=N_CORES)
    xt = nc.dram_tensor("xt", [IN_DIM, B], F16, kind="ExternalInput").ap()
    idxa = nc.dram_tensor("idxa", [P, OUT_DIM // 16], I16,
                          kind="ExternalInput").ap()
    idxb = nc.dram_tensor("idxb", [P, OUT_DIM // 16], I16,
                          kind="ExternalInput").ap()
    c0d = nc.dram_tensor("c0", [P, M], F32, kind="ExternalInput").ap()
    cad = nc.dram_tensor("ca", [P, M], F32, kind="ExternalInput").ap()
    cbd = nc.dram_tensor("cb", [P, M], F32, kind="ExternalInput").ap()
    cabd = nc.dram_tensor("cab", [P, M], F32, kind="ExternalInput").ap()
    yt = nc.dram_tensor("yt", [OUT_DIM, B], F16, kind="ExternalOutput").ap()

    mult = mybir.AluOpType.mult
    add = mybir.AluOpType.add
    ident_f = mybir.ActivationFunctionType.Identity

    with tile.TileContext(nc) as tc:
        with tc.tile_pool(name="const", bufs=1) as cpool:
            ia_t = cpool.tile([P, OUT_DIM // 16], I16, tag="ia")
            nc.sync.dma_start(ia_t[:], idxa)
            ib_t = cpool.tile([P, OUT_DIM // 16], I16, tag="ib")
            nc.sync.dma_start(ib_t[:], idxb)
            c0_t = cpool.tile([P, M], F32, tag="c0")
            nc.sync.dma_start(c0_t[:], c0d)
            ca_t = cpool.tile([P, M], F32, tag="ca")
            nc.sync.dma_start(ca_t[:], cad)
            cb_t = cpool.tile([P, M], F32, tag="cb")
            nc.sync.dma_start(cb_t[:], cbd)
            cab_t = cpool.tile([P, M], F32, tag="cab")
            nc.sync.dma_start(cab_t[:], cabd)

            with tc.tile_pool(name="gth", bufs=3) as gp, \
                 tc.tile_pool(name="tmp", bufs=3) as tp, \
                 tc.tile_pool(name="ysp", bufs=16) as ysp:
                # chunk sizes in 128-col slots; small first chunks
                # let compute start while later gathers stream
                sizes = [1, 3] + [SLOTS] * (NCH - 2) + [3, 1]
                base = 0
                for ns in sizes:
                    n = ns * P
                    i0, i1 = base * 8, base * 8 + n // 16
                    at = gp.tile([P, SLOTS, B], F16, tag="a")
                    nc.gpsimd.dma_gather(
                        at[:, :ns, :], xt, ia_t[:, i0:i1],
                        n, n, B, elem_step=B)
                    bt = gp.tile([P, SLOTS, B], F16, tag="b")
                    nc.gpsimd.dma_gather(
                        bt[:, :ns, :], xt, ib_t[:, i0:i1],
                        n, n, B, elem_step=B)
                    for s in range(ns):
                        m = base + s
                        a_s = at[:, s, :]
                        b_s = bt[:, s, :]
                        # u = cab*b + ca ; v = cb*b + c0 -- one on
                        # DVE (fused tensor_scalar), one on ACT,
                        # alternating per slot for engine balance
                        u = tp.tile([P, B], F16, tag="u")
                        v = tp.tile([P, B], F16, tag="v")
                        if m % 2 == 0:
                            nc.vector.tensor_scalar(
                                u[:], b_s, cab_t[:, m:m + 1],
                                ca_t[:, m:m + 1], mult, add)
                            nc.scalar.activation(
                                v[:], b_s, ident_f,
                                bias=c0_t[:, m:m + 1],
                                scale=cb_t[:, m:m + 1])
                        else:
                            nc.scalar.activation(
                                u[:], b_s, ident_f,
                                bias=ca_t[:, m:m + 1],
                                scale=cab_t[:, m:m + 1])
                            nc.vector.tensor_scalar(
                                v[:], b_s, cb_t[:, m:m + 1],
                                c0_t[:, m:m + 1], mult, add)
                        # w = u*a ; ys = w + v  (DVE, fp16 2x)
                        w = tp.tile([P, B], F16, tag="w")
                        nc.vector.tensor_mul(w[:], u[:], a_s)
                        ys = ysp.tile([P, B], F16, tag="ys")
                        nc.vector.tensor_add(ys[:], w[:], v[:])
                        # out-dim-major writeback: 4 KiB per partition
                        nc.sync.dma_start(yt[m * P:(m + 1) * P, :], ys[:])
                    base += ns
    nc.compile()
    return nc


def _wrap_idx(idx):
    """[4096] int -> [128, 256] int16: index j sits at partition j%16
    (replicated over the 8 16-partition groups), column j//16."""
    idx = np.asarray(idx).astype(np.int64)
    out = idx.reshape(OUT_DIM // 16, 16).T.astype(np.int16)  # [16, 256]
    return np.ascontiguousarray(np.tile(out, (8, 1)))


def _coef_pt(col):
    """[4096] -> [128, 32] f32 with [p, m] = col[m*128 + p]."""
    return np.ascontiguousarray(col.reshape(M, P).T.astype(np.float32))


def kernel(x, weights, idx_a, idx_b, trace=False):
    global LAST_EXEC_NS
    x = np.asarray(x, dtype=np.float32)
    weights = np.asarray(weights, dtype=np.float64)
    idx_a = np.asarray(idx_a)
    idx_b = np.asarray(idx_b)

    # host: coef table (tiny: [4096, 16] softmax @ [16, 4])
    wmax = weights.max(axis=-1, keepdims=True)
    e = np.exp(weights - wmax)
    wprob = e / e.sum(axis=-1, keepdims=True)
    coef = (wprob @ GATE_COEFS)  # [4096, 4] float64, cols (c0, ca, cb, cab)

    ia_w = _wrap_idx(idx_a)
    ib_w = _wrap_idx(idx_b)
    c0 = _coef_pt(coef[:, 0])
    ca = _coef_pt(coef[:, 1])
    cb = _coef_pt(coef[:, 2])
    cab = _coef_pt(coef[:, 3])

    # per-core transposed fp16 x shard [IN_DIM, B]
    x16 = x.astype(np.float16)

    if "nc" not in _NC_CACHE:
        _NC_CACHE["nc"] = _build_nc()
    nc = _NC_CACHE["nc"]

    in_maps = []
    for i in range(N_CORES):
        in_maps.append({
            "xt": np.ascontiguousarray(x16[i * B:(i + 1) * B, :].T),
            "idxa": ia_w, "idxb": ib_w,
            "c0": c0, "ca": ca, "cb": cb, "cab": cab,
        })
    res = run_bass_kernel_spmd(nc, in_maps, core_ids=list(range(N_CORES)),
                               trace=trace)
    LAST_EXEC_NS = res.exec_time_ns
    # unshard: each core's yT [OUT_DIM, B] fp16 -> batch-major f32
    y = np.empty((BATCH, OUT_DIM), dtype=np.float32)
    for i in range(N_CORES):
        y[i * B:(i + 1) * B, :] = res.results[i]["yt"].T
    return y


# revision 19
# speedup vs baseline: 1.0067x; 1.0067x over previous
"""Trainium2 Bass kernel for the difflogic LogicLayer problem.

Computation: y = c0 + ca*a + cb*b + cab*a*b where a = x[:, idx_a],
b = x[:, idx_b] and (c0, ca, cb, cab) = softmax(weights) @ GATE_COEFS.

Strategy (8-core SPMD, data-parallel over batch):
  - Host: compute the tiny [4096, 16] softmax -> [4096, 4] coef table,
    marshal the per-core x shard as a transposed fp16 table
    xT [4096, 2048] (shard layout choice), wrap the index lists into the
    16-partition dma_gather layout. The per-core output shard is
    likewise out-dim-major yT [4096, 2048] fp16; the host unshards by
    transposing back to batch-major f32.
  - Device, per core (single pipeline):
      For each chunk of 512 output columns:
        * dma_gather rows idx_a/idx_b from DRAM xT (4 KiB fp16 rows,
          near line rate) -> out-dim-major tiles [128, 4, 2048].
        * Per 128-col slot: fused DVE tensor_scalar u = cab*b + ca,
          ACT affine v = cb*b + c0, DVE w = u*a, ys = w + v (fp16).
        * DMA ys -> yT rows (4 KiB contiguous per partition).
  HBM traffic/core: 33.5 MiB gather read + 16.8 MiB yT write.
"""
import numpy as np

import concourse.bacc as bacc
import concourse.mybir as mybir
import concourse.tile as tile
from concourse.bass_utils import run_bass_kernel_spmd

# difflogic gate coefficients: rows = gates, cols = (const, a, b, ab)
GATE_COEFS = np.array([
    [0, 0, 0, 0], [0, 0, 0, 1], [0, 1, 0, -1], [0, 1, 0, 0],
    [0, 0, 1, -1], [0, 0, 1, 0], [0, 1, 1, -2], [0, 1, 1, -1],
    [1, -1, -1, 1], [1, -1, -1, 2], [1, 0, -1, 0], [1, 0, -1, 1],
    [1, -1, 0, 0], [1, -1, 0, 1], [1, 0, 0, -1], [1, 0, 0, 0],
], dtype=np.float64)  # [16, 4]

N_CORES = 8
P = 128
BATCH = 16384
IN_DIM = 4096
OUT_DIM = 4096
B = BATCH // N_CORES          # 2048 rows per core
CHUNK = 512                   # indices per dma_gather
NCH = OUT_DIM // CHUNK        # 8 chunks
SLOTS = CHUNK // P            # 4 slots per chunk
M = OUT_DIM // P              # 32 col blocks

F32 = mybir.dt.float32
F16 = mybir.dt.float16
I16 = mybir.dt.int16

LAST_EXEC_NS = None
_NC_CACHE = {}


def _build_nc():
    nc = bacc.Bacc("TRN2", target_bir_lowering=False, debug=False,
                   num_devices=N_CORES)
    xt = nc.dram_tensor("xt", [IN_DIM, B], F16, kind="ExternalInput").ap()
    idxa = nc.dram_tensor("idxa", [P, OUT_DIM // 16], I16,
                          kind="ExternalInput").ap()
    idxb = nc.dram_tensor("idxb", [P, OUT_DIM // 16], I16,
                          kind="ExternalInput").ap()
    c0d = nc.dram_tensor("c0", [P, M], F32, kind="ExternalInput").ap()
    cad = nc.dram_tensor("ca", [P, M], F32, kind="ExternalInput").ap()
    cbd = nc.dram_tensor("cb", [P, M], F32, kind="ExternalInput").ap()
    cabd = nc.dram_tensor("cab", [P, M], F32, kind="ExternalInput").ap()
    yt = nc.dram_tensor("yt", [OUT_DIM, B], F16, kind="ExternalOutput").ap()

    mult = mybir.AluOpType.mult
    add = mybir.AluOpType.add
    ident_f = mybir.ActivationFunctionType.Identity

    with tile.TileContext(nc) as tc:
        with tc.tile_pool(name="const", bufs=1) as cpool:
            ia_t = cpool.tile([P, OUT_DIM // 16], I16, tag="ia")
            nc.sync.dma_start(ia_t[:], idxa)
            ib_t = cpool.tile([P, OUT_DIM // 16], I16, tag="ib")
            nc.sync.dma_start(ib_t[:], idxb)
            c0_t = cpool.tile([P, M], F32, tag="c0")
            nc.sync.dma_start(c0_t[:], c0d)
            ca_t = cpool.tile([P, M], F32, tag="ca")
            nc.sync.dma_start(ca_t[:], cad)
            cb_t = cpool.tile([P, M], F32, tag="cb")
            nc.sync.dma_start(cb_t[:], cbd)
            cab_t = cpool.tile([P, M], F32, tag="cab")
            nc.sync.dma_start(cab_t[:], cabd)

            with tc.tile_pool(name="gth", bufs=3) as gp, \
                 tc.tile_pool(name="tmp", bufs=3) as tp, \
                 tc.tile_pool(name="ysp", bufs=16) as ysp:
                # chunk sizes in 128-col slots; small first chunks
                # let compute start while later gathers stream
                sizes = [1, 3] + [SLOTS] * (NCH - 2) + [3, 1]
                base = 0
                for ns in sizes:
                    n = ns * P
                    i0, i1 = base * 8, base * 8 + n // 16
                    at = gp.tile([P, SLOTS, B], F16, tag="a")
                    nc.gpsimd.dma_gather(
                        at[:, :ns, :], xt, ia_t[:, i0:i1],
                        n, n, B, elem_step=B)
                    bt = gp.tile([P, SLOTS, B], F16, tag="b")
                    nc.gpsimd.dma_gather(
                        bt[:, :ns, :], xt, ib_t[:, i0:i1],
                        n, n, B, elem_step=B)
                    for s in range(ns):
                        m = base + s
                        a_s = at[:, s, :]
                        b_s = bt[:, s, :]
                        # u = cab*b + ca ; v = cb*b + c0 -- one on
                        # DVE (fused tensor_scalar), one on ACT,
                        # alternating per slot for engine balance
                        u = tp.tile([P, B], F16, tag="u")
                        v = tp.tile([P, B], F16, tag="v")
                        if s % 2 == 0:
                            nc.vector.tensor_scalar(
                                u[:], b_s, cab_t[:, m:m + 1],
                                ca_t[:, m:m + 1], mult, add)
                            nc.scalar.activation(
                                v[:], b_s, ident_f,
                                bias=c0_t[:, m:m + 1],
                                scale=cb_t[:, m:m + 1])
                        else:
                            nc.scalar.activation(
                                u[:], b_s, ident_f,
                                bias=ca_t[:, m:m + 1],
                                scale=cab_t[:, m:m + 1])
                            nc.vector.tensor_scalar(
                                v[:], b_s, cb_t[:, m:m + 1],
                                c0_t[:, m:m + 1], mult, add)
                        # w = u*a ; ys = w + v  (DVE, fp16 2x)
                        w = tp.tile([P, B], F16, tag="w")
                        nc.vector.tensor_mul(w[:], u[:], a_s)
                        ys = ysp.tile([P, B], F16, tag="ys")
                        nc.vector.tensor_add(ys[:], w[:], v[:])
                        # out-dim-major writeback: 4 KiB per partition
                        nc.sync.dma_start(yt[m * P:(m + 1) * P, :], ys[:])
                    base += ns
    nc.compile()
    return nc


def _wrap_idx(idx):
    """[4096] int -> [128, 256] int16: index j sits at partition j%16
    (replicated over the 8 16-partition groups), column j//16."""
    idx = np.asarray(idx).astype(np.int64)
    out = idx.reshape(OUT_DIM // 16, 16).T.astype(np.int16)  # [16, 256]
    return np.ascontiguousarray(np.tile(out, (8, 1)))


def _coef_pt(col):
    """[4096] -> [128, 32] f32 with [p, m] = col[m*128 + p]."""
    return np.ascontiguousarray(col.reshape(M, P).T.astype(np.float32))


def kernel(x, weights, idx_a, idx_b, trace=False):
    global LAST_EXEC_NS
    x = np.asarray(x, dtype=np.float32)
    weights = np.asarray(weights, dtype=np.float64)
    idx_a = np.asarray(idx_a)
    idx_b = np.asarray(idx_b)

    # host: coef table (tiny: [4096, 16] softmax @ [16, 4])
    wmax = weights.max(axis=-1, keepdims=True)
    e = np.exp(weights - wmax)
    wprob = e / e.sum(axis=-1, keepdims=True)
    coef = (wprob @ GATE_COEFS)  # [4096, 4] float64, cols (c0, ca, cb, cab)

    ia_w = _wrap_idx(idx_a)
    ib_w = _wrap_idx(idx_b)
    c0 = _coef_pt(coef[:, 0])
    ca = _coef_pt(coef[:, 1])
    cb = _coef_pt(coef[:, 2])
    cab = _coef_pt(coef[:, 3])

    # per-core transposed fp16 x shard [IN_DIM, B]
    x16 = x.astype(np.float16)

    if "nc" not in _NC_CACHE:
        _NC_CACHE["nc"] = _build_nc()
    nc = _NC_CACHE["nc"]

    in_maps = []
    for i in range(N_CORES):
        in_maps.append({
            "xt": np.ascontiguousarray(x16[i * B:(i + 1) * B, :].T),
            "idxa": ia_w, "idxb": ib_w,
            "c0": c0, "ca": ca, "cb": cb, "cab": cab,
        })
    res = run_bass_kernel_spmd(nc, in_maps, core_ids=list(range(N_CORES)),
                               trace=trace)
    LAST_EXEC_NS = res.exec_time_ns
    # unshard: each core's yT [OUT_DIM, B] fp16 -> batch-major f32
    y = np.empty((BATCH, OUT_DIM), dtype=np.float32)
    for i in range(N_CORES):
        y[i * B:(i + 1) * B, :] = res.results[i]["yt"].T
    return y


# revision 21
# speedup vs baseline: 1.0422x; 1.0352x over previous
"""Trainium2 Bass kernel for the difflogic LogicLayer problem.

Computation: y = c0 + ca*a + cb*b + cab*a*b where a = x[:, idx_a],
b = x[:, idx_b] and (c0, ca, cb, cab) = softmax(weights) @ GATE_COEFS.

Strategy (8-core SPMD, data-parallel over batch):
  - Host: compute the tiny [4096, 16] softmax -> [4096, 4] coef table,
    marshal the per-core x shard as a transposed fp16 table
    xT [4096, 2048] (shard layout choice), wrap the index lists into the
    16-partition dma_gather layout. The per-core output shard is
    likewise out-dim-major yT [4096, 2048] fp16; the host unshards by
    transposing back to batch-major f32.
  - Device, per core (single pipeline):
      For each chunk of 512 output columns:
        * dma_gather rows idx_a/idx_b from DRAM xT (4 KiB fp16 rows,
          near line rate) -> out-dim-major tiles [128, 4, 2048].
        * Per 128-col slot: fused DVE tensor_scalar u = cab*b + ca,
          ACT affine v = cb*b + c0, DVE w = u*a, ys = w + v (fp16).
        * DMA ys -> yT rows (4 KiB contiguous per partition).
  HBM traffic/core: 33.5 MiB gather read + 16.8 MiB yT write.
"""
import numpy as np

import concourse.bacc as bacc
import concourse.mybir as mybir
import concourse.tile as tile
from concourse.bass_utils import run_bass_kernel_spmd

# difflogic gate coefficients: rows = gates, cols = (const, a, b, ab)
GATE_COEFS = np.array([
    [0, 0, 0, 0], [0, 0, 0, 1], [0, 1, 0, -1], [0, 1, 0, 0],
    [0, 0, 1, -1], [0, 0, 1, 0], [0, 1, 1, -2], [0, 1, 1, -1],
    [1, -1, -1, 1], [1, -1, -1, 2], [1, 0, -1, 0], [1, 0, -1, 1],
    [1, -1, 0, 0], [1, -1, 0, 1], [1, 0, 0, -1], [1, 0, 0, 0],
], dtype=np.float64)  # [16, 4]

N_CORES = 8
P = 128
BATCH = 16384
IN_DIM = 4096
OUT_DIM = 4096
B = BATCH // N_CORES          # 2048 rows per core
CHUNK = 512                   # indices per dma_gather
NCH = OUT_DIM // CHUNK        # 8 chunks
SLOTS = CHUNK // P            # 4 slots per chunk
M = OUT_DIM // P              # 32 col blocks

F32 = mybir.dt.float32
F16 = mybir.dt.float16
I16 = mybir.dt.int16

LAST_EXEC_NS = None
_NC_CACHE = {}


def _build_nc():
    nc = bacc.Bacc("TRN2", target_bir_lowering=False, debug=False,
                   num_devices=N_CORES)
    xt = nc.dram_tensor("xt", [IN_DIM, B], F16, kind="ExternalInput").ap()
    idxa = nc.dram_tensor("idxa", [P, OUT_DIM // 16], I16,
                          kind="ExternalInput").ap()
    idxb = nc.dram_tensor("idxb", [P, OUT_DIM // 16], I16,
                          kind="ExternalInput").ap()
    c0d = nc.dram_tensor("c0", [P, M], F32, kind="ExternalInput").ap()
    cad = nc.dram_tensor("ca", [P, M], F32, kind="ExternalInput").ap()
    cbd = nc.dram_tensor("cb", [P, M], F32, kind="ExternalInput").ap()
    cabd = nc.dram_tensor("cab", [P, M], F32, kind="ExternalInput").ap()
    yt = nc.dram_tensor("yt", [OUT_DIM, B], F16, kind="ExternalOutput").ap()

    mult = mybir.AluOpType.mult
    add = mybir.AluOpType.add
    ident_f = mybir.ActivationFunctionType.Identity

    with tile.TileContext(nc) as tc:
        with tc.tile_pool(name="const", bufs=1) as cpool:
            ia_t = cpool.tile([P, OUT_DIM // 16], I16, tag="ia")
            nc.sync.dma_start(ia_t[:], idxa)
            ib_t = cpool.tile([P, OUT_DIM // 16], I16, tag="ib")
            nc.sync.dma_start(ib_t[:], idxb)
            c0_t = cpool.tile([P, M], F32, tag="c0")
            nc.sync.dma_start(c0_t[:], c0d)
            ca_t = cpool.tile([P, M], F32, tag="ca")
            nc.sync.dma_start(ca_t[:], cad)
            cb_t = cpool.tile([P, M], F32, tag="cb")
            nc.sync.dma_start(cb_t[:], cbd)
            cab_t = cpool.tile([P, M], F32, tag="cab")
            nc.sync.dma_start(cab_t[:], cabd)

            with tc.tile_pool(name="gth", bufs=3) as gp, \
                 tc.tile_pool(name="tmp", bufs=2) as tp, \
                 tc.tile_pool(name="ysp", bufs=7) as ysp:
                # chunk sizes in 128-col slots; small first chunks
                # let compute start while later gathers stream
                sizes = [1, 3] + [SLOTS] * (NCH - 2) + [3, 1]
                base = 0
                for ns in sizes:
                    n = ns * P
                    i0, i1 = base * 8, base * 8 + n // 16
                    bt = gp.tile([P, SLOTS, B], F16, tag="b")
                    nc.gpsimd.dma_gather(
                        bt[:, :ns, :], xt, ib_t[:, i0:i1],
                        n, n, B, elem_step=B)
                    at = gp.tile([P, SLOTS, B], F16, tag="a")
                    nc.gpsimd.dma_gather(
                        at[:, :ns, :], xt, ia_t[:, i0:i1],
                        n, n, B, elem_step=B)
                    # process slots in pairs: affines per slot
                    # (alternating DVE tensor_scalar / ACT activation),
                    # then paired [P, 2, B] TT ops and one 2 MiB store
                    s = 0
                    while s < ns:
                        k = 2 if s + 1 < ns else 1
                        m = base + s
                        u = tp.tile([P, 2, B], F16, tag="u")
                        v = tp.tile([P, 2, B], F16, tag="v")
                        for j in range(k):
                            mj = m + j
                            b_s = bt[:, s + j, :]
                            if (s + j) % 2 == 0:
                                nc.vector.tensor_scalar(
                                    u[:, j, :], b_s, cab_t[:, mj:mj + 1],
                                    ca_t[:, mj:mj + 1], mult, add)
                                nc.scalar.activation(
                                    v[:, j, :], b_s, ident_f,
                                    bias=c0_t[:, mj:mj + 1],
                                    scale=cb_t[:, mj:mj + 1])
                            else:
                                nc.scalar.activation(
                                    u[:, j, :], b_s, ident_f,
                                    bias=ca_t[:, mj:mj + 1],
                                    scale=cab_t[:, mj:mj + 1])
                                nc.vector.tensor_scalar(
                                    v[:, j, :], b_s, cb_t[:, mj:mj + 1],
                                    c0_t[:, mj:mj + 1], mult, add)
                        # w = u*a ; ys = w + v  (DVE, fp16 2x)
                        w = tp.tile([P, 2, B], F16, tag="w")
                        nc.vector.tensor_mul(
                            w[:, :k, :], u[:, :k, :], at[:, s:s + k, :])
                        ys = ysp.tile([P, 2, B], F16, tag="ys")
                        nc.vector.tensor_add(
                            ys[:, :k, :], w[:, :k, :], v[:, :k, :])
                        # out-dim-major writeback: 4 KiB per partition
                        dsty = yt[m * P:(m + k) * P, :].rearrange(
                            "(j p) b -> p j b", j=k)
                        nc.sync.dma_start(dsty, ys[:, :k, :])
                        s += k
                    base += ns
    nc.compile()
    return nc


def _wrap_idx(idx):
    """[4096] int -> [128, 256] int16: index j sits at partition j%16
    (replicated over the 8 16-partition groups), column j//16."""
    idx = np.asarray(idx).astype(np.int64)
    out = idx.reshape(OUT_DIM // 16, 16).T.astype(np.int16)  # [16, 256]
    return np.ascontiguousarray(np.tile(out, (8, 1)))


def _coef_pt(col):
    """[4096] -> [128, 32] f32 with [p, m] = col[m*128 + p]."""
    return np.ascontiguousarray(col.reshape(M, P).T.astype(np.float32))


def kernel(x, weights, idx_a, idx_b, trace=False):
    global LAST_EXEC_NS
    x = np.asarray(x, dtype=np.float32)
    weights = np.asarray(weights, dtype=np.float64)
    idx_a = np.asarray(idx_a)
    idx_b = np.asarray(idx_b)

    # host: coef table (tiny: [4096, 16] softmax @ [16, 4])
    wmax = weights.max(axis=-1, keepdims=True)
    e = np.exp(weights - wmax)
    wprob = e / e.sum(axis=-1, keepdims=True)
    coef = (wprob @ GATE_COEFS)  # [4096, 4] float64, cols (c0, ca, cb, cab)

    ia_w = _wrap_idx(idx_a)
    ib_w = _wrap_idx(idx_b)
    c0 = _coef_pt(coef[:, 0])
    ca = _coef_pt(coef[:, 1])
    cb = _coef_pt(coef[:, 2])
    cab = _coef_pt(coef[:, 3])

    # per-core transposed fp16 x shard [IN_DIM, B]
    x16 = x.astype(np.float16)

    if "nc" not in _NC_CACHE:
        _NC_CACHE["nc"] = _build_nc()
    nc = _NC_CACHE["nc"]

    in_maps = []
    for i in range(N_CORES):
        in_maps.append({
            "xt": np.ascontiguousarray(x16[i * B:(i + 1) * B, :].T),
            "idxa": ia_w, "idxb": ib_w,
            "c0": c0, "ca": ca, "cb": cb, "cab": cab,
        })
    res = run_bass_kernel_spmd(nc, in_maps, core_ids=list(range(N_CORES)),
                               trace=trace)
    LAST_EXEC_NS = res.exec_time_ns
    # unshard: each core's yT [OUT_DIM, B] fp16 -> batch-major f32
    y = np.empty((BATCH, OUT_DIM), dtype=np.float32)
    for i in range(N_CORES):
        y[i * B:(i + 1) * B, :] = res.results[i]["yt"].T
    return y
